# revision 10
# baseline (speedup 1.0000x reference)
"""Distributed CLIP loss on 8 Trainium2 NeuronCores (Bass/Tile).

Strategy (data-parallel over image rows, per the distributed-CLIP pattern):
  - Core i owns image rows [2048*i, 2048*(i+1)).  It receives its image shard
    transposed (d-major, fp8-e4m3, pre-scaled by 8) plus the FULL text matrix
    transposed and *rolled* by -2048*i rows, so the diagonal block of the
    logits always lands in local columns [0, 2048) — every core runs the
    identical program.
  - On device, each core computes its (2048 x 16384) block of
    E = exp(scale' * img8 @ txt8^T + bias) tile-by-tile:
      * PE matmul in fp8 DoubleRow mode (256-deep contraction per matmul,
        fp32 PSUM accumulation over 3 chunks of the 768-dim contraction)
      * ScalarE exp over a 4-bank (2048-wide) PSUM span, writing bf16 E
        tiles and accumulating the per-row sums (fused accum_out)
      * VectorE reduces the bf16 E tiles at 2x packed rate:
        quad-batched column sum / column max accumulators (4 row-tiles per
        instruction) and segmented row-max partials, plus the masked
        diagonal extraction
  - The host finishes: partition/core reductions of colsum/colmax,
    log-sum-exp assembly, the two CE means, and the argmax==label accuracies
    via (max == diag) equality in bf16-E-domain (all values come from the
    same device tiles, so equality is bit-faithful).

The fp8 quantization perturbs each logit by ~sigma 0.02; the loss tolerance
(2e-2 relative on ~9.8) leaves orders of magnitude of headroom, and the
accuracy equality tests have a >0.5 logit margin on this input distribution
(verified host-side against the exact reference).
"""

import math

import ml_dtypes
import numpy as np

import bass_rust
import concourse.bass as bass
import concourse.tile as tile
from concourse import mybir
from concourse.bass_utils import run_bass_kernel_spmd
from concourse.vector_clock import ScopedClock

N_CORES = 8
B = 16384
D = 768
BL = B // N_CORES          # 2048 local image rows per core
N_RT = BL // 128           # 16 row tiles of 128 rows
W = 2048                   # column-group width (4 PSUM banks)
N_G = B // W               # 8 column groups
N_C = D // 256             # 3 DoubleRow contraction chunks (256 each)
QUAD = 4                   # row-tiles batched per DVE accumulate op
FP8 = mybir.dt.float8e4
BF16 = mybir.dt.bfloat16
F32 = mybir.dt.float32
SF = 8.0                   # fp8 pre-scale on both feature matrices

_MAXW = 1  # this walrus build allows a single sync-wait per CTRL instruction


def _patched_drain_and_barrier(self, tick_clock, wait_clock):
    """Tail drain with its waits split one-per-instruction (walrus limit)."""
    nc = self.nc
    drain_inst = nc.sync.drain()
    wait_clock.add_sem_waits(
        drain_inst.ins, ScopedClock({None: tick_clock.global_clock})
    )
    si = drain_inst.ins.sync_info
    waits = list(si.on_wait or [])
    if len(waits) > _MAXW:
        si.on_wait = waits[:_MAXW]
        rest = waits[_MAXW:]
        for i in range(0, len(rest), _MAXW):
            extra = nc.sync.drain()
            extra.ins.sync_info = bass_rust.SyncInfo(
                on_wait=rest[i : i + _MAXW], on_update=[]
            )
    nc.all_engine_barrier()
    assert self.sems is not None
    popped = nc._tile_sem_poison_stack.pop()
    assert popped is self._sem_poison
    nc.clear_and_free_semaphores(list(self.sems.allocated().values()))
    nc.all_engine_barrier()


tile.TileContext._drain_and_barrier = _patched_drain_and_barrier

_orig_lower_ordered_insts = tile.TileContext._lower_ordered_insts


def _patched_lower_ordered_insts(self, ordered):
    """Split multi-wait instructions: this walrus build allows one sync-wait
    per ISA instruction, so carry the extras on same-engine NOPs in front."""
    nc = self.nc
    for bb_name, insts in ordered.items():
        new_insts = []
        for inst in insts:
            si = inst.sync_info
            if (
                si is not None
                and si.on_wait
                and len(si.on_wait) > _MAXW
                and inst.engine != mybir.EngineType.Unassigned
            ):
                waits = list(si.on_wait)
                si.on_wait = waits[-_MAXW:]
                carry = waits[: -_MAXW]
                for i in range(0, len(carry), _MAXW):
                    nop = mybir.InstNoOp(
                        name=nc.get_next_instruction_name(),
                        engine=inst.engine,
                        ins=[],
                        outs=[],
                        sync_info=bass_rust.SyncInfo(
                            on_wait=carry[i : i + _MAXW], on_update=[]
                        ),
                    )
                    new_insts.append(nop)
            new_insts.append(inst)
        ordered[bb_name] = new_insts
    return _orig_lower_ordered_insts(self, ordered)


tile.TileContext._lower_ordered_insts = _patched_lower_ordered_insts


def _dedup_ldweights(nc) -> int:
    """Remove back-to-back InstLdweights that reload identical weights.

    tile_legalize pairs every matmul with its own LDWEIGHTS even when 4
    consecutive matmuls share the same stationary tile.  Removal is safe ONLY
    because the weights tiles here (img_sb) are written once and never
    overwritten, so the PE array state stays valid across the elided reloads.
    LDWs carrying any sync wait/update are kept (their sem bookkeeping must
    not change), and any other PE instruction resets the tracking.
    """
    removed = 0
    for f in nc.m.functions:
        for bb in f.blocks:
            insts = list(bb.instructions)
            keep = []
            last_key = None
            changed = False
            for ins in insts:
                tn = type(ins).__name__
                if tn == "InstLdweights":
                    si = ins.sync_info
                    clean = si is None or (not si.on_wait and not si.on_update)
                    key = (
                        str(ins.ins[0]),
                        str(ins.is_transpose),
                        str(getattr(ins, "perf_mode", None)),
                        str(getattr(ins, "tile_position", None)),
                    )
                    if clean and key == last_key:
                        removed += 1
                        changed = True
                        continue
                    last_key = key
                elif tn == "InstMatmult":
                    pass  # matmuls leave the loaded weights untouched
                elif getattr(ins, "engine", None) == mybir.EngineType.PE:
                    last_key = None  # unknown PE op: stop eliding
                keep.append(ins)
            if changed:
                bb.instructions = keep
    return removed


def build_program(scale: float, bias: float, reps: int = 1) -> bass.Bass:
    """Build the per-core Bass program (identical on all 8 cores).

    scale/bias are the activation affine: E = exp(scale * raw_dot + bias),
    where raw_dot is the fp8 matmul output (features pre-scaled by SF, so
    scale already carries the 1/SF^2 factor).

    reps > 1 repeats the whole computation for slope-based timing."""
    nc = bass.Bass("TRN2", target_bir_lowering=False, debug=False)

    imgT = nc.dram_tensor("imgT", (D, BL), FP8, kind="ExternalInput").ap()
    txtT = nc.dram_tensor("txtT", (D, B), FP8, kind="ExternalInput").ap()
    ident = nc.dram_tensor("ident", (128, QUAD, 32), BF16, kind="ExternalInput").ap()

    colsum_d = nc.dram_tensor(
        "colsum", (N_G, 128, QUAD, W), BF16, kind="ExternalOutput"
    ).ap()
    colmax_d = nc.dram_tensor(
        "colmax", (N_G, 128, QUAD, W), BF16, kind="ExternalOutput"
    ).ap()
    zrow_d = nc.dram_tensor(
        "zrow", (128, N_RT * N_G), F32, kind="ExternalOutput"
    ).ap()
    rowmax_d = nc.dram_tensor(
        "rowmax", (128, N_RT, N_G * 32), BF16, kind="ExternalOutput"
    ).ap()
    diag_d = nc.dram_tensor("diag", (128, N_RT), BF16, kind="ExternalOutput").ap()

    EXP = mybir.ActivationFunctionType.Exp
    X = mybir.AxisListType.X
    XY = mybir.AxisListType.XY
    DR = mybir.MatmulPerfMode.DoubleRow
    NQ = N_RT // QUAD          # 4 quads of row tiles

    with tile.TileContext(nc) as tc:
        with tc.tile_pool(name="const", bufs=1) as constp, \
             tc.tile_pool(name="imgp", bufs=1) as imgp, \
             tc.tile_pool(name="txtp", bufs=2) as txtp, \
             tc.tile_pool(name="psum", bufs=2, space="PSUM") as psump, \
             tc.tile_pool(name="ep", bufs=4) as ep, \
             tc.tile_pool(name="accs", bufs=3) as accp, \
             tc.tile_pool(name="stats", bufs=1) as statp, \
             tc.tile_pool(name="dscr", bufs=2) as dscrp:

            ident_sb = constp.tile([128, QUAD, 32], BF16)
            nc.sync.dma_start(ident_sb[:], ident)

            # img weights: [p, c, i, m] with contraction d = 256c + 128i + p
            img_sb = imgp.tile([128, N_C, 2, BL], FP8)
            for c in range(N_C):
                for i in range(2):
                    nc.sync.dma_start(
                        img_sb[:, c, i, :],
                        imgT[256 * c + 128 * i : 256 * c + 128 * i + 128, :],
                    )

            # fused row sums from the exp ops, one slot per (rt, g)
            rowsum_slots = statp.tile([128, N_RT * N_G], F32)
            # row-max tree partials: 32 per (rt, g)
            rowmax_slots = statp.tile([128, N_RT, N_G * 32], BF16)
            diag_sb = statp.tile([128, N_RT], BF16)

            for rep in range(reps):
              for g in range(N_G):
                txt_g = txtp.tile(
                    [128, N_C, 2, W], FP8, tag="txt_g", name=f"txt_{rep}_{g}"
                )
                for c in range(N_C):
                    for i in range(2):
                        nc.sync.dma_start(
                            txt_g[:, c, i, :],
                            txtT[
                                256 * c + 128 * i : 256 * c + 128 * i + 128,
                                g * W : (g + 1) * W,
                            ],
                        )
                # quad-lane accumulators; lanes merged in place at group end
                cs4 = accp.tile([128, QUAD, W], BF16, tag="cs")
                cm4 = accp.tile([128, QUAD, W], BF16, tag="cm")
                for q in range(NQ):
                    e_q = ep.tile([128, QUAD, 64, 32], BF16, tag="e")
                    for sq in range(QUAD):
                        rt = q * QUAD + sq
                        pb = psump.tile(
                            [128, 64, 32], F32, tag="pb", name=f"pb{g}_{rt}"
                        )
                        lhsT = img_sb[:, :, :, rt * 128 : (rt + 1) * 128]
                        for c in range(N_C):
                            for b in range(4):
                                nc.tensor.matmul(
                                    pb[:, b * 16 : (b + 1) * 16, :],
                                    lhsT[:, c, :, :],
                                    txt_g[:, c, :, b * 512 : (b + 1) * 512],
                                    start=(c == 0),
                                    stop=(c == N_C - 1),
                                    perf_mode=DR,
                                )
                        s = rt * N_G + g
                        nc.scalar.activation(
                            out=e_q[:, sq, :, :],
                            in_=pb[:],
                            func=EXP,
                            scale=scale,
                            bias=bias,
                            accum_out=rowsum_slots[:, s : s + 1],
                        )
                        if g == 0:
                            scr = dscrp.tile([128, QUAD, 32], BF16, tag="scr")
                            nc.vector.tensor_mul(
                                scr[:],
                                e_q[:, sq, 4 * rt : 4 * rt + 4, :],
                                ident_sb[:],
                            )
                            nc.vector.reduce_max(
                                out=diag_sb[:, rt : rt + 1], in_=scr[:], axis=XY
                            )
                    # quad-batched accumulate over the 4 fresh row tiles.
                    # q==0 has no accumulator op: at q==1 the two quads
                    # combine directly (saves a copy per accumulator), so
                    # quad 0's row-max tree (which clobbers its tile) is
                    # deferred until after that combine.
                    if q == 1:
                        e_p = prev_e
                        nc.vector.tensor_add(cs4[:], e_p[:], e_q[:])
                        nc.vector.tensor_max(cm4[:], e_p[:], e_q[:])
                    elif q > 1:
                        nc.vector.tensor_add(cs4[:], cs4[:], e_q[:])
                        nc.vector.tensor_max(cm4[:], cm4[:], e_q[:])
                    # row-max via in-place TT-max tree (2x packed rate; the
                    # 1x tensor_reduce path costs twice as much).  Clobbers
                    # the tile, which has no further readers by then.
                    def rowmax_tree(e_t, qi):
                        k = 64
                        while k > 2:
                            nc.vector.tensor_max(
                                e_t[:, :, 0 : k // 2, :],
                                e_t[:, :, 0 : k // 2, :],
                                e_t[:, :, k // 2 : k, :],
                            )
                            k //= 2
                        nc.vector.tensor_max(
                            rowmax_slots[
                                :,
                                qi * QUAD : (qi + 1) * QUAD,
                                g * 32 : (g + 1) * 32,
                            ],
                            e_t[:, :, 0:1, :],
                            e_t[:, :, 1:2, :],
                        )

                    if q == 0:
                        prev_e = e_q
                    elif q == 1:
                        rowmax_tree(prev_e, 0)
                        rowmax_tree(e_q, 1)
                    else:
                        rowmax_tree(e_q, q)
                # lanes are merged host-side; ship all four
                nc.sync.dma_start(colsum_d[g], cs4[:])
                nc.sync.dma_start(colmax_d[g], cm4[:])
                if g == 0:
                    # diag complete after the first (diagonal) group
                    nc.sync.dma_start(diag_d, diag_sb[:])

            # per-(rt, g) partials reduce on the host
            nc.sync.dma_start(zrow_d, rowsum_slots[:])
            nc.sync.dma_start(rowmax_d, rowmax_slots[:])

    _dedup_ldweights(nc)
    return nc


def prepare_inputs(image_features, text_features):
    """Host-side sharding: fp8 cast (pre-scaled), transposes, per-core roll."""
    img = np.asarray(image_features, dtype=np.float32)
    txt = np.asarray(text_features, dtype=np.float32)
    img8 = (img * SF).astype(ml_dtypes.float8_e4m3)
    txt8 = (txt * SF).astype(ml_dtypes.float8_e4m3)
    imgT_full = np.ascontiguousarray(img8.T)      # (D, B)
    txtT_full = np.ascontiguousarray(txt8.T)      # (D, B)
    ident = np.eye(128, dtype=ml_dtypes.bfloat16).reshape(128, QUAD, 32)
    in_maps = []
    for i in range(N_CORES):
        imgT_i = np.ascontiguousarray(imgT_full[:, i * BL : (i + 1) * BL])
        txtT_i = np.roll(txtT_full, -BL * i, axis=1)
        in_maps.append({"imgT": imgT_i, "txtT": txtT_i, "ident": ident})
    return in_maps


def postprocess(results):
    """Host-side gather/reduce of the per-core stats -> (loss, accs)."""
    zrow = np.empty(B, dtype=np.float64)
    rowmax = np.empty(B, dtype=np.float64)
    diag = np.empty(B, dtype=np.float64)
    zcol = np.zeros(B, dtype=np.float64)
    colmax = np.full(B, -np.inf, dtype=np.float64)
    for i, r in enumerate(results):
        # (128, 16*8) f32 / (128, 16, 256) bf16 -> local row index 128*rt + p
        zr = r["zrow"].astype(np.float64).reshape(128, N_RT, N_G).sum(axis=2)
        rm = r["rowmax"].astype(np.float64).max(axis=2)
        zrow[i * BL : (i + 1) * BL] = zr.T.reshape(-1)
        rowmax[i * BL : (i + 1) * BL] = rm.T.reshape(-1)
        diag[i * BL : (i + 1) * BL] = r["diag"].T.reshape(-1).astype(np.float64)
        # (8, 128, 4, 2048): local (rolled) col 2048*g + c; partial over
        # partitions and quad lanes
        cs = r["colsum"].astype(np.float64).sum(axis=(1, 2)).reshape(-1)
        cm = r["colmax"].astype(np.float64).max(axis=(1, 2)).reshape(-1)
        # local col 0 corresponds to global col 2048*i (text was rolled by -2048*i)
        zcol += np.roll(cs, BL * i)
        colmax = np.maximum(colmax, np.roll(cm, BL * i))

    loss_i2t = np.mean(np.log(zrow) - np.log(diag))
    loss_t2i = np.mean(np.log(zcol) - np.log(diag))
    loss = (loss_i2t + loss_t2i) / 2.0
    i2t_acc = np.mean(rowmax == diag)
    t2i_acc = np.mean(colmax == diag)
    return (
        np.float32(loss),
        np.float32(i2t_acc),
        np.float32(t2i_acc),
    )


_program_cache: dict[tuple[float, float], bass.Bass] = {}


def get_program(scale: float, bias: float) -> bass.Bass:
    key = (scale, bias)
    if key not in _program_cache:
        _program_cache[key] = build_program(scale, bias)
    return _program_cache[key]


def compute_scale_bias(image_features, text_features, logit_scale):
    ls = float(np.asarray(logit_scale))
    scale = 100.0 if ls >= math.log(100.0) else float(math.exp(ls))
    # |logits| <= scale * max|img_i| * max|txt_j|; keep exp argument <= ~70
    # so f32 never overflows even for unnormalized inputs.
    img = np.asarray(image_features, dtype=np.float32)
    txt = np.asarray(text_features, dtype=np.float32)
    ni = float(np.sqrt((img.astype(np.float64) ** 2).sum(axis=1).max()))
    nt = float(np.sqrt((txt.astype(np.float64) ** 2).sum(axis=1).max()))
    bound = scale * ni * nt
    bias = -max(0.0, bound - 70.0)
    # fold the fp8 pre-scale into the activation affine
    return scale / (SF * SF), bias


def kernel(image_features, text_features, logit_scale):
    scale, bias = compute_scale_bias(image_features, text_features, logit_scale)
    nc = get_program(scale, bias)
    in_maps = prepare_inputs(image_features, text_features)
    try:
        res = run_bass_kernel_spmd(nc, in_maps, core_ids=list(range(N_CORES)))
    except Exception:
        # transient accelerator hiccups have been observed on this relay;
        # one retry on a fresh attempt usually clears them
        import time as _time

        _time.sleep(2.0)
        res = run_bass_kernel_spmd(nc, in_maps, core_ids=list(range(N_CORES)))
    return postprocess(res.results)


# revision 14
# speedup vs baseline: 1.0048x; 1.0048x over previous
"""Distributed CLIP loss on 8 Trainium2 NeuronCores (Bass/Tile).

Strategy (data-parallel over image rows, per the distributed-CLIP pattern):
  - Core i owns image rows [2048*i, 2048*(i+1)).  It receives its image shard
    transposed (d-major, fp8-e4m3, pre-scaled by 8) plus the FULL text matrix
    transposed and *rolled* by -2048*i rows, so the diagonal block of the
    logits always lands in local columns [0, 2048) — every core runs the
    identical program.
  - On device, each core computes its (2048 x 16384) block of
    E = exp(scale' * img8 @ txt8^T + bias) tile-by-tile:
      * PE matmul in fp8 DoubleRow mode (256-deep contraction per matmul,
        fp32 PSUM accumulation over 3 chunks of the 768-dim contraction)
      * ScalarE exp over a 4-bank (2048-wide) PSUM span, writing bf16 E
        tiles and accumulating the per-row sums (fused accum_out)
      * VectorE reduces the bf16 E tiles at 2x packed rate:
        quad-batched column sum / column max accumulators (4 row-tiles per
        instruction) and segmented row-max partials, plus the masked
        diagonal extraction
  - The host finishes: partition/core reductions of colsum/colmax,
    log-sum-exp assembly, the two CE means, and the argmax==label accuracies
    via (max == diag) equality in bf16-E-domain (all values come from the
    same device tiles, so equality is bit-faithful).

The fp8 quantization perturbs each logit by ~sigma 0.02; the loss tolerance
(2e-2 relative on ~9.8) leaves orders of magnitude of headroom, and the
accuracy equality tests have a >0.5 logit margin on this input distribution
(verified host-side against the exact reference).
"""

import math

import ml_dtypes
import numpy as np

import bass_rust
import concourse.bass as bass
import concourse.tile as tile
from concourse import mybir
from concourse.bass_utils import run_bass_kernel_spmd
from concourse.vector_clock import ScopedClock

N_CORES = 8
B = 16384
D = 768
BL = B // N_CORES          # 2048 local image rows per core
N_RT = BL // 128           # 16 row tiles of 128 rows
W = 2048                   # column-group width (4 PSUM banks)
N_G = B // W               # 8 column groups
N_C = D // 256             # 3 DoubleRow contraction chunks (256 each)
QUAD = 4                   # row-tiles batched per DVE accumulate op
FP8 = mybir.dt.float8e4
BF16 = mybir.dt.bfloat16
F32 = mybir.dt.float32
SF = 8.0                   # fp8 pre-scale on both feature matrices

_MAXW = 1  # this walrus build allows a single sync-wait per CTRL instruction


def _patched_drain_and_barrier(self, tick_clock, wait_clock):
    """Tail drain with its waits split one-per-instruction (walrus limit)."""
    nc = self.nc
    drain_inst = nc.sync.drain()
    wait_clock.add_sem_waits(
        drain_inst.ins, ScopedClock({None: tick_clock.global_clock})
    )
    si = drain_inst.ins.sync_info
    waits = list(si.on_wait or [])
    if len(waits) > _MAXW:
        si.on_wait = waits[:_MAXW]
        rest = waits[_MAXW:]
        for i in range(0, len(rest), _MAXW):
            extra = nc.sync.drain()
            extra.ins.sync_info = bass_rust.SyncInfo(
                on_wait=rest[i : i + _MAXW], on_update=[]
            )
    nc.all_engine_barrier()
    assert self.sems is not None
    popped = nc._tile_sem_poison_stack.pop()
    assert popped is self._sem_poison
    nc.clear_and_free_semaphores(list(self.sems.allocated().values()))
    nc.all_engine_barrier()


tile.TileContext._drain_and_barrier = _patched_drain_and_barrier

_orig_lower_ordered_insts = tile.TileContext._lower_ordered_insts


def _patched_lower_ordered_insts(self, ordered):
    """Split multi-wait instructions: this walrus build allows one sync-wait
    per ISA instruction, so carry the extras on same-engine NOPs in front."""
    nc = self.nc
    for bb_name, insts in ordered.items():
        new_insts = []
        for inst in insts:
            si = inst.sync_info
            if (
                si is not None
                and si.on_wait
                and len(si.on_wait) > _MAXW
                and inst.engine != mybir.EngineType.Unassigned
            ):
                waits = list(si.on_wait)
                si.on_wait = waits[-_MAXW:]
                carry = waits[: -_MAXW]
                for i in range(0, len(carry), _MAXW):
                    nop = mybir.InstNoOp(
                        name=nc.get_next_instruction_name(),
                        engine=inst.engine,
                        ins=[],
                        outs=[],
                        sync_info=bass_rust.SyncInfo(
                            on_wait=carry[i : i + _MAXW], on_update=[]
                        ),
                    )
                    new_insts.append(nop)
            new_insts.append(inst)
        ordered[bb_name] = new_insts
    return _orig_lower_ordered_insts(self, ordered)


tile.TileContext._lower_ordered_insts = _patched_lower_ordered_insts


def _dedup_ldweights(nc) -> int:
    """Remove back-to-back InstLdweights that reload identical weights.

    tile_legalize pairs every matmul with its own LDWEIGHTS even when 4
    consecutive matmuls share the same stationary tile.  Removal is safe ONLY
    because the weights tiles here (img_sb) are written once and never
    overwritten, so the PE array state stays valid across the elided reloads.
    LDWs carrying any sync wait/update are kept (their sem bookkeeping must
    not change), and any other PE instruction resets the tracking.
    """
    removed = 0
    for f in nc.m.functions:
        for bb in f.blocks:
            insts = list(bb.instructions)
            keep = []
            last_key = None
            changed = False
            for ins in insts:
                tn = type(ins).__name__
                if tn == "InstLdweights":
                    si = ins.sync_info
                    clean = si is None or (not si.on_wait and not si.on_update)
                    key = (
                        str(ins.ins[0]),
                        str(ins.is_transpose),
                        str(getattr(ins, "perf_mode", None)),
                        str(getattr(ins, "tile_position", None)),
                    )
                    if clean and key == last_key:
                        removed += 1
                        changed = True
                        continue
                    last_key = key
                elif tn == "InstMatmult":
                    pass  # matmuls leave the loaded weights untouched
                elif getattr(ins, "engine", None) == mybir.EngineType.PE:
                    last_key = None  # unknown PE op: stop eliding
                keep.append(ins)
            if changed:
                bb.instructions = keep
    return removed


def build_program(scale: float, bias: float, reps: int = 1) -> bass.Bass:
    """Build the per-core Bass program (identical on all 8 cores).

    scale/bias are the activation affine: E = exp(scale * raw_dot + bias),
    where raw_dot is the fp8 matmul output (features pre-scaled by SF, so
    scale already carries the 1/SF^2 factor).

    reps > 1 repeats the whole computation for slope-based timing."""
    nc = bass.Bass("TRN2", target_bir_lowering=False, debug=False)

    imgT = nc.dram_tensor("imgT", (D, BL), FP8, kind="ExternalInput").ap()
    txtT = nc.dram_tensor("txtT", (D, B), FP8, kind="ExternalInput").ap()
    ident = nc.dram_tensor("ident", (128, QUAD, 32), BF16, kind="ExternalInput").ap()

    colsum_d = nc.dram_tensor(
        "colsum", (N_G, 128, QUAD, W), BF16, kind="ExternalOutput"
    ).ap()
    colmax_d = nc.dram_tensor(
        "colmax", (N_G, 128, QUAD, W), BF16, kind="ExternalOutput"
    ).ap()
    zrow_d = nc.dram_tensor(
        "zrow", (128, N_RT * N_G), F32, kind="ExternalOutput"
    ).ap()
    rowmax_d = nc.dram_tensor(
        "rowmax", (128, N_RT, N_G * 32), BF16, kind="ExternalOutput"
    ).ap()
    diag_d = nc.dram_tensor("diag", (128, N_RT), BF16, kind="ExternalOutput").ap()

    EXP = mybir.ActivationFunctionType.Exp
    X = mybir.AxisListType.X
    XY = mybir.AxisListType.XY
    DR = mybir.MatmulPerfMode.DoubleRow
    NQ = N_RT // QUAD          # 4 quads of row tiles

    with tile.TileContext(nc) as tc:
        with tc.tile_pool(name="const", bufs=1) as constp, \
             tc.tile_pool(name="imgp", bufs=1) as imgp, \
             tc.tile_pool(name="txtp", bufs=2) as txtp, \
             tc.tile_pool(name="psum", bufs=2, space="PSUM") as psump, \
             tc.tile_pool(name="ep", bufs=5) as ep, \
             tc.tile_pool(name="accs", bufs=2) as accp, \
             tc.tile_pool(name="stats", bufs=1) as statp, \
             tc.tile_pool(name="dscr", bufs=2) as dscrp:

            ident_sb = constp.tile([128, QUAD, 32], BF16)
            nc.scalar.dma_start(ident_sb[:], ident)

            # img weights: [p, c, i, m] with contraction d = 256c + 128i + p.
            # Startup DMAs rotate across otherwise-idle engine queues and
            # interleave img/txt chunks c-major so the c=0 matmuls can
            # issue as early as possible (the sync queue alone costs
            # ~650ns dispatch per DMA, serializing ~10us before first MM).
            img_sb = imgp.tile([128, N_C, 2, BL], FP8)

            # fused row sums from the exp ops, one slot per (rt, g)
            rowsum_slots = statp.tile([128, N_RT * N_G], F32)
            # row-max tree partials: 32 per (rt, g)
            rowmax_slots = statp.tile([128, N_RT, N_G * 32], BF16)
            diag_sb = statp.tile([128, N_RT], BF16)

            for rep in range(reps):
              for g in range(N_G):
                txt_g = txtp.tile(
                    [128, N_C, 2, W], FP8, tag="txt_g", name=f"txt_{rep}_{g}"
                )
                first = rep == 0 and g == 0
                queues = [nc.sync, nc.scalar]
                qi = 0
                for c in range(N_C):
                    for i in range(2):
                        if first:
                            nc.gpsimd.dma_start(
                                img_sb[:, c, i, :],
                                imgT[
                                    256 * c + 128 * i : 256 * c + 128 * i + 128, :
                                ],
                            )
                        eng = queues[qi % len(queues)] if first else nc.sync
                        qi += 1
                        eng.dma_start(
                            txt_g[:, c, i, :],
                            txtT[
                                256 * c + 128 * i : 256 * c + 128 * i + 128,
                                g * W : (g + 1) * W,
                            ],
                        )
                # quad-lane accumulators; lanes merged in place at group end
                cs4 = accp.tile([128, QUAD, W], BF16, tag="cs")
                cm4 = accp.tile([128, QUAD, W], BF16, tag="cm")
                for q in range(NQ):
                    e_q = ep.tile([128, QUAD, 64, 32], BF16, tag="e")
                    for sq in range(QUAD):
                        rt = q * QUAD + sq
                        pb = psump.tile(
                            [128, 64, 32], F32, tag="pb", name=f"pb{g}_{rt}"
                        )
                        lhsT = img_sb[:, :, :, rt * 128 : (rt + 1) * 128]
                        for c in range(N_C):
                            for b in range(4):
                                nc.tensor.matmul(
                                    pb[:, b * 16 : (b + 1) * 16, :],
                                    lhsT[:, c, :, :],
                                    txt_g[:, c, :, b * 512 : (b + 1) * 512],
                                    start=(c == 0),
                                    stop=(c == N_C - 1),
                                    perf_mode=DR,
                                )
                        s = rt * N_G + g
                        nc.scalar.activation(
                            out=e_q[:, sq, :, :],
                            in_=pb[:],
                            func=EXP,
                            scale=scale,
                            bias=bias,
                            accum_out=rowsum_slots[:, s : s + 1],
                        )
                        if g == 0:
                            scr = dscrp.tile([128, QUAD, 32], BF16, tag="scr")
                            nc.vector.tensor_mul(
                                scr[:],
                                e_q[:, sq, 4 * rt : 4 * rt + 4, :],
                                ident_sb[:],
                            )
                            nc.vector.reduce_max(
                                out=diag_sb[:, rt : rt + 1], in_=scr[:], axis=XY
                            )
                    # quad-batched accumulate over the 4 fresh row tiles.
                    # q==0 has no accumulator op: at q==1 the two quads
                    # combine directly (saves a copy per accumulator), so
                    # quad 0's row-max tree (which clobbers its tile) is
                    # deferred until after that combine.
                    if q == 1:
                        e_p = prev_e
                        nc.vector.tensor_add(cs4[:], e_p[:], e_q[:])
                        nc.vector.tensor_max(cm4[:], e_p[:], e_q[:])
                    elif q > 1:
                        nc.vector.tensor_add(cs4[:], cs4[:], e_q[:])
                        nc.vector.tensor_max(cm4[:], cm4[:], e_q[:])
                    # row-max via in-place TT-max tree (2x packed rate; the
                    # 1x tensor_reduce path costs twice as much).  Clobbers
                    # the tile, which has no further readers by then.
                    def rowmax_tree(e_t, qi):
                        k = 64
                        while k > 2:
                            nc.vector.tensor_max(
                                e_t[:, :, 0 : k // 2, :],
                                e_t[:, :, 0 : k // 2, :],
                                e_t[:, :, k // 2 : k, :],
                            )
                            k //= 2
                        nc.vector.tensor_max(
                            rowmax_slots[
                                :,
                                qi * QUAD : (qi + 1) * QUAD,
                                g * 32 : (g + 1) * 32,
                            ],
                            e_t[:, :, 0:1, :],
                            e_t[:, :, 1:2, :],
                        )

                    if q == 0:
                        prev_e = e_q
                    elif q == 1:
                        rowmax_tree(prev_e, 0)
                        rowmax_tree(e_q, 1)
                    else:
                        rowmax_tree(e_q, q)
                # lanes are merged host-side; ship all four
                nc.sync.dma_start(colsum_d[g], cs4[:])
                nc.sync.dma_start(colmax_d[g], cm4[:])
                if g == 0:
                    # diag complete after the first (diagonal) group
                    nc.sync.dma_start(diag_d, diag_sb[:])

            # per-(rt, g) partials reduce on the host
            nc.sync.dma_start(zrow_d, rowsum_slots[:])
            nc.sync.dma_start(rowmax_d, rowmax_slots[:])

    _dedup_ldweights(nc)
    return nc


def prepare_inputs(image_features, text_features):
    """Host-side sharding: fp8 cast (pre-scaled), transposes, per-core roll."""
    img = np.asarray(image_features, dtype=np.float32)
    txt = np.asarray(text_features, dtype=np.float32)
    img8 = (img * SF).astype(ml_dtypes.float8_e4m3)
    txt8 = (txt * SF).astype(ml_dtypes.float8_e4m3)
    imgT_full = np.ascontiguousarray(img8.T)      # (D, B)
    txtT_full = np.ascontiguousarray(txt8.T)      # (D, B)
    ident = np.eye(128, dtype=ml_dtypes.bfloat16).reshape(128, QUAD, 32)
    in_maps = []
    for i in range(N_CORES):
        imgT_i = np.ascontiguousarray(imgT_full[:, i * BL : (i + 1) * BL])
        txtT_i = np.roll(txtT_full, -BL * i, axis=1)
        in_maps.append({"imgT": imgT_i, "txtT": txtT_i, "ident": ident})
    return in_maps


def postprocess(results):
    """Host-side gather/reduce of the per-core stats -> (loss, accs)."""
    zrow = np.empty(B, dtype=np.float64)
    rowmax = np.empty(B, dtype=np.float64)
    diag = np.empty(B, dtype=np.float64)
    zcol = np.zeros(B, dtype=np.float64)
    colmax = np.full(B, -np.inf, dtype=np.float64)
    for i, r in enumerate(results):
        # (128, 16*8) f32 / (128, 16, 256) bf16 -> local row index 128*rt + p
        zr = r["zrow"].astype(np.float64).reshape(128, N_RT, N_G).sum(axis=2)
        rm = r["rowmax"].astype(np.float64).max(axis=2)
        zrow[i * BL : (i + 1) * BL] = zr.T.reshape(-1)
        rowmax[i * BL : (i + 1) * BL] = rm.T.reshape(-1)
        diag[i * BL : (i + 1) * BL] = r["diag"].T.reshape(-1).astype(np.float64)
        # (8, 128, 4, 2048): local (rolled) col 2048*g + c; partial over
        # partitions and quad lanes
        cs = r["colsum"].astype(np.float64).sum(axis=(1, 2)).reshape(-1)
        cm = r["colmax"].astype(np.float64).max(axis=(1, 2)).reshape(-1)
        # local col 0 corresponds to global col 2048*i (text was rolled by -2048*i)
        zcol += np.roll(cs, BL * i)
        colmax = np.maximum(colmax, np.roll(cm, BL * i))

    loss_i2t = np.mean(np.log(zrow) - np.log(diag))
    loss_t2i = np.mean(np.log(zcol) - np.log(diag))
    loss = (loss_i2t + loss_t2i) / 2.0
    i2t_acc = np.mean(rowmax == diag)
    t2i_acc = np.mean(colmax == diag)
    return (
        np.float32(loss),
        np.float32(i2t_acc),
        np.float32(t2i_acc),
    )


_program_cache: dict[tuple[float, float], bass.Bass] = {}


def get_program(scale: float, bias: float) -> bass.Bass:
    key = (scale, bias)
    if key not in _program_cache:
        _program_cache[key] = build_program(scale, bias)
    return _program_cache[key]


def compute_scale_bias(image_features, text_features, logit_scale):
    ls = float(np.asarray(logit_scale))
    scale = 100.0 if ls >= math.log(100.0) else float(math.exp(ls))
    # |logits| <= scale * max|img_i| * max|txt_j|; keep exp argument <= ~70
    # so f32 never overflows even for unnormalized inputs.
    img = np.asarray(image_features, dtype=np.float32)
    txt = np.asarray(text_features, dtype=np.float32)
    ni = float(np.sqrt((img.astype(np.float64) ** 2).sum(axis=1).max()))
    nt = float(np.sqrt((txt.astype(np.float64) ** 2).sum(axis=1).max()))
    bound = scale * ni * nt
    bias = -max(0.0, bound - 70.0)
    # fold the fp8 pre-scale into the activation affine
    return scale / (SF * SF), bias


def kernel(image_features, text_features, logit_scale):
    scale, bias = compute_scale_bias(image_features, text_features, logit_scale)
    nc = get_program(scale, bias)
    in_maps = prepare_inputs(image_features, text_features)
    try:
        res = run_bass_kernel_spmd(nc, in_maps, core_ids=list(range(N_CORES)))
    except Exception:
        # transient accelerator hiccups have been observed on this relay;
        # one retry on a fresh attempt usually clears them
        import time as _time

        _time.sleep(2.0)
        res = run_bass_kernel_spmd(nc, in_maps, core_ids=list(range(N_CORES)))
    return postprocess(res.results)


# revision 16
# speedup vs baseline: 1.0832x; 1.0781x over previous
"""Distributed CLIP loss on 8 Trainium2 NeuronCores (Bass/Tile).

Strategy (data-parallel over image rows, per the distributed-CLIP pattern):
  - Core i owns image rows [2048*i, 2048*(i+1)).  It receives its image shard
    transposed (d-major, fp8-e4m3, pre-scaled by 8) plus the FULL text matrix
    transposed and *rolled* by -2048*i rows, so the diagonal block of the
    logits always lands in local columns [0, 2048) — every core runs the
    identical program.
  - On device, each core computes its (2048 x 16384) block of
    E = exp(scale' * img8 @ txt8^T + bias) tile-by-tile:
      * PE matmul in fp8 DoubleRow mode (256-deep contraction per matmul,
        fp32 PSUM accumulation over 3 chunks of the 768-dim contraction)
      * ScalarE exp over a 4-bank (2048-wide) PSUM span, writing bf16 E
        tiles and accumulating the per-row sums (fused accum_out)
      * VectorE reduces the bf16 E tiles at 2x packed rate:
        quad-batched column sum / column max accumulators (4 row-tiles per
        instruction) and segmented row-max partials, plus the masked
        diagonal extraction
  - The host finishes: partition/core reductions of colsum/colmax,
    log-sum-exp assembly, the two CE means, and the argmax==label accuracies
    via (max == diag) equality in bf16-E-domain (all values come from the
    same device tiles, so equality is bit-faithful).

The fp8 quantization perturbs each logit by ~sigma 0.02; the loss tolerance
(2e-2 relative on ~9.8) leaves orders of magnitude of headroom, and the
accuracy equality tests have a >0.5 logit margin on this input distribution
(verified host-side against the exact reference).
"""

import math

import ml_dtypes
import numpy as np

import bass_rust
import concourse.bass as bass
import concourse.tile as tile
from concourse import mybir
from concourse.bass_utils import run_bass_kernel_spmd
from concourse.vector_clock import ScopedClock

N_CORES = 8
B = 16384
D = 768
BL = B // N_CORES          # 2048 local image rows per core
N_RT = BL // 128           # 16 row tiles of 128 rows
W = 2048                   # column-group width (4 PSUM banks)
N_G = B // W               # 8 column groups
N_C = D // 256             # 3 DoubleRow contraction chunks (256 each)
QUAD = 4                   # row-tiles batched per DVE accumulate op
FP8 = mybir.dt.float8e4
BF16 = mybir.dt.bfloat16
F32 = mybir.dt.float32
SF = 8.0                   # fp8 pre-scale on both feature matrices

_MAXW = 1  # this walrus build allows a single sync-wait per CTRL instruction


def _patched_drain_and_barrier(self, tick_clock, wait_clock):
    """Tail drain with its waits split one-per-instruction (walrus limit)."""
    nc = self.nc
    drain_inst = nc.sync.drain()
    wait_clock.add_sem_waits(
        drain_inst.ins, ScopedClock({None: tick_clock.global_clock})
    )
    si = drain_inst.ins.sync_info
    waits = list(si.on_wait or [])
    if len(waits) > _MAXW:
        si.on_wait = waits[:_MAXW]
        rest = waits[_MAXW:]
        for i in range(0, len(rest), _MAXW):
            extra = nc.sync.drain()
            extra.ins.sync_info = bass_rust.SyncInfo(
                on_wait=rest[i : i + _MAXW], on_update=[]
            )
    nc.all_engine_barrier()
    assert self.sems is not None
    popped = nc._tile_sem_poison_stack.pop()
    assert popped is self._sem_poison
    nc.clear_and_free_semaphores(list(self.sems.allocated().values()))
    nc.all_engine_barrier()


tile.TileContext._drain_and_barrier = _patched_drain_and_barrier

_orig_lower_ordered_insts = tile.TileContext._lower_ordered_insts


def _patched_lower_ordered_insts(self, ordered):
    """Split multi-wait instructions: this walrus build allows one sync-wait
    per ISA instruction, so carry the extras on same-engine NOPs in front."""
    nc = self.nc
    for bb_name, insts in ordered.items():
        new_insts = []
        for inst in insts:
            si = inst.sync_info
            if (
                si is not None
                and si.on_wait
                and len(si.on_wait) > _MAXW
                and inst.engine != mybir.EngineType.Unassigned
            ):
                waits = list(si.on_wait)
                si.on_wait = waits[-_MAXW:]
                carry = waits[: -_MAXW]
                for i in range(0, len(carry), _MAXW):
                    nop = mybir.InstNoOp(
                        name=nc.get_next_instruction_name(),
                        engine=inst.engine,
                        ins=[],
                        outs=[],
                        sync_info=bass_rust.SyncInfo(
                            on_wait=carry[i : i + _MAXW], on_update=[]
                        ),
                    )
                    new_insts.append(nop)
            new_insts.append(inst)
        ordered[bb_name] = new_insts
    return _orig_lower_ordered_insts(self, ordered)


tile.TileContext._lower_ordered_insts = _patched_lower_ordered_insts


def _dedup_ldweights(nc) -> int:
    """Remove back-to-back InstLdweights that reload identical weights.

    tile_legalize pairs every matmul with its own LDWEIGHTS even when 4
    consecutive matmuls share the same stationary tile.  Removal is safe ONLY
    because the weights tiles here (img_sb) are written once and never
    overwritten, so the PE array state stays valid across the elided reloads.
    LDWs carrying any sync wait/update are kept (their sem bookkeeping must
    not change), and any other PE instruction resets the tracking.
    """
    removed = 0
    for f in nc.m.functions:
        for bb in f.blocks:
            insts = list(bb.instructions)
            keep = []
            last_key = None
            changed = False
            for ins in insts:
                tn = type(ins).__name__
                if tn == "InstLdweights":
                    si = ins.sync_info
                    clean = si is None or (not si.on_wait and not si.on_update)
                    key = (
                        str(ins.ins[0]),
                        str(ins.is_transpose),
                        str(getattr(ins, "perf_mode", None)),
                        str(getattr(ins, "tile_position", None)),
                    )
                    if clean and key == last_key:
                        removed += 1
                        changed = True
                        continue
                    last_key = key
                elif tn == "InstMatmult":
                    pass  # matmuls leave the loaded weights untouched
                elif getattr(ins, "engine", None) == mybir.EngineType.PE:
                    last_key = None  # unknown PE op: stop eliding
                keep.append(ins)
            if changed:
                bb.instructions = keep
    return removed


def build_program(scale: float, bias: float, reps: int = 1) -> bass.Bass:
    """Build the per-core Bass program (identical on all 8 cores).

    scale/bias are the activation affine: E = exp(scale * raw_dot + bias),
    where raw_dot is the fp8 matmul output (features pre-scaled by SF, so
    scale already carries the 1/SF^2 factor).

    reps > 1 repeats the whole computation for slope-based timing."""
    nc = bass.Bass("TRN2", target_bir_lowering=False, debug=False)

    imgT = nc.dram_tensor("imgT", (D, BL), FP8, kind="ExternalInput").ap()
    txtT = nc.dram_tensor("txtT", (D, B), FP8, kind="ExternalInput").ap()
    ident = nc.dram_tensor("ident", (128, QUAD, 32), BF16, kind="ExternalInput").ap()

    colsum_d = nc.dram_tensor(
        "colsum", (N_G, 128, 2 * QUAD, W), BF16, kind="ExternalOutput"
    ).ap()
    colmax_d = nc.dram_tensor(
        "colmax", (N_G, 128, 2 * QUAD, W), BF16, kind="ExternalOutput"
    ).ap()
    zrow_d = nc.dram_tensor(
        "zrow", (128, N_RT * N_G), F32, kind="ExternalOutput"
    ).ap()
    rowmax_d = nc.dram_tensor(
        "rowmax", (128, N_RT, N_G * 32), BF16, kind="ExternalOutput"
    ).ap()
    diag_d = nc.dram_tensor("diag", (128, N_RT), BF16, kind="ExternalOutput").ap()

    EXP = mybir.ActivationFunctionType.Exp
    X = mybir.AxisListType.X
    XY = mybir.AxisListType.XY
    DR = mybir.MatmulPerfMode.DoubleRow
    NQ = N_RT // QUAD          # 4 quads of row tiles

    with tile.TileContext(nc) as tc:
        with tc.tile_pool(name="const", bufs=1) as constp, \
             tc.tile_pool(name="imgp", bufs=1) as imgp, \
             tc.tile_pool(name="txtp", bufs=2) as txtp, \
             tc.tile_pool(name="psum", bufs=2, space="PSUM") as psump, \
             tc.tile_pool(name="ep", bufs=4) as ep, \
             tc.tile_pool(name="accs", bufs=1) as accp, \
             tc.tile_pool(name="stats", bufs=1) as statp, \
             tc.tile_pool(name="dscr", bufs=2) as dscrp:

            ident_sb = constp.tile([128, QUAD, 32], BF16)
            nc.scalar.dma_start(ident_sb[:], ident)

            # img weights: [p, c, i, m] with contraction d = 256c + 128i + p.
            # Startup DMAs rotate across otherwise-idle engine queues and
            # interleave img/txt chunks c-major so the c=0 matmuls can
            # issue as early as possible (the sync queue alone costs
            # ~650ns dispatch per DMA, serializing ~10us before first MM).
            img_sb = imgp.tile([128, N_C, 2, BL], FP8)

            # fused row sums from the exp ops, one slot per (rt, g)
            rowsum_slots = statp.tile([128, N_RT * N_G], F32)
            # row-max tree partials: 32 per (rt, g)
            rowmax_slots = statp.tile([128, N_RT, N_G * 32], BF16)
            diag_sb = statp.tile([128, N_RT], BF16)

            for rep in range(reps):
              for g in range(N_G):
                txt_g = txtp.tile(
                    [128, N_C, 2, W], FP8, tag="txt_g", name=f"txt_{rep}_{g}"
                )
                first = rep == 0 and g == 0
                queues = [nc.sync, nc.scalar]
                qi = 0
                for c in range(N_C):
                    for i in range(2):
                        if first:
                            nc.gpsimd.dma_start(
                                img_sb[:, c, i, :],
                                imgT[
                                    256 * c + 128 * i : 256 * c + 128 * i + 128, :
                                ],
                            )
                        eng = queues[qi % len(queues)] if first else nc.sync
                        qi += 1
                        eng.dma_start(
                            txt_g[:, c, i, :],
                            txtT[
                                256 * c + 128 * i : 256 * c + 128 * i + 128,
                                g * W : (g + 1) * W,
                            ],
                        )
                # 8-lane accumulators: each quad-pair combines once into its
                # own lane block (no read-modify-write chain across quads);
                # the host merges all 8 lanes with the partition reduction.
                cs8 = accp.tile([128, 2 * QUAD, W], BF16, tag="cs")
                cm8 = accp.tile([128, 2 * QUAD, W], BF16, tag="cm")
                for q in range(NQ):
                    e_q = ep.tile([128, QUAD, 64, 32], BF16, tag="e")
                    for sq in range(QUAD):
                        rt = q * QUAD + sq
                        pb = psump.tile(
                            [128, 64, 32], F32, tag="pb", name=f"pb{g}_{rt}"
                        )
                        lhsT = img_sb[:, :, :, rt * 128 : (rt + 1) * 128]
                        for c in range(N_C):
                            for b in range(4):
                                nc.tensor.matmul(
                                    pb[:, b * 16 : (b + 1) * 16, :],
                                    lhsT[:, c, :, :],
                                    txt_g[:, c, :, b * 512 : (b + 1) * 512],
                                    start=(c == 0),
                                    stop=(c == N_C - 1),
                                    perf_mode=DR,
                                )
                        s = rt * N_G + g
                        nc.scalar.activation(
                            out=e_q[:, sq, :, :],
                            in_=pb[:],
                            func=EXP,
                            scale=scale,
                            bias=bias,
                            accum_out=rowsum_slots[:, s : s + 1],
                        )
                        if g == 0:
                            scr = dscrp.tile([128, QUAD, 32], BF16, tag="scr")
                            nc.vector.tensor_mul(
                                scr[:],
                                e_q[:, sq, 4 * rt : 4 * rt + 4, :],
                                ident_sb[:],
                            )
                            nc.vector.reduce_max(
                                out=diag_sb[:, rt : rt + 1], in_=scr[:], axis=XY
                            )
                    # pair-combine into this pair's own lane block: even
                    # quads just park their tile; odd quads emit one add and
                    # one max over both (their trees are deferred likewise).
                    if q % 2 == 1:
                        e_p = prev_e
                        lo, hi = (q // 2) * QUAD, (q // 2 + 1) * QUAD
                        nc.vector.tensor_add(cs8[:, lo:hi, :], e_p[:], e_q[:])
                        nc.vector.tensor_max(cm8[:, lo:hi, :], e_p[:], e_q[:])
                        # ship each half as soon as it exists so the single
                        # accumulator buffer is WAR-free by the next group
                        nc.sync.dma_start(colsum_d[g, :, lo:hi, :], cs8[:, lo:hi, :])
                        nc.sync.dma_start(colmax_d[g, :, lo:hi, :], cm8[:, lo:hi, :])
                    # row-max via in-place TT-max tree (2x packed rate; the
                    # 1x tensor_reduce path costs twice as much).  Clobbers
                    # the tile, which has no further readers by then.
                    def rowmax_tree(e_t, qi):
                        k = 64
                        while k > 2:
                            nc.vector.tensor_max(
                                e_t[:, :, 0 : k // 2, :],
                                e_t[:, :, 0 : k // 2, :],
                                e_t[:, :, k // 2 : k, :],
                            )
                            k //= 2
                        nc.vector.tensor_max(
                            rowmax_slots[
                                :,
                                qi * QUAD : (qi + 1) * QUAD,
                                g * 32 : (g + 1) * 32,
                            ],
                            e_t[:, :, 0:1, :],
                            e_t[:, :, 1:2, :],
                        )

                    if q % 2 == 0:
                        prev_e = e_q
                    else:
                        rowmax_tree(prev_e, q - 1)
                        rowmax_tree(e_q, q)
                # lanes are merged host-side; ship all four
                if g == 0:
                    # diag complete after the first (diagonal) group
                    nc.sync.dma_start(diag_d, diag_sb[:])

            # per-(rt, g) partials reduce on the host
            nc.sync.dma_start(zrow_d, rowsum_slots[:])
            nc.sync.dma_start(rowmax_d, rowmax_slots[:])

    _dedup_ldweights(nc)
    return nc


def prepare_inputs(image_features, text_features):
    """Host-side sharding: fp8 cast (pre-scaled), transposes, per-core roll."""
    img = np.asarray(image_features, dtype=np.float32)
    txt = np.asarray(text_features, dtype=np.float32)
    img8 = (img * SF).astype(ml_dtypes.float8_e4m3)
    txt8 = (txt * SF).astype(ml_dtypes.float8_e4m3)
    imgT_full = np.ascontiguousarray(img8.T)      # (D, B)
    txtT_full = np.ascontiguousarray(txt8.T)      # (D, B)
    ident = np.eye(128, dtype=ml_dtypes.bfloat16).reshape(128, QUAD, 32)
    in_maps = []
    for i in range(N_CORES):
        imgT_i = np.ascontiguousarray(imgT_full[:, i * BL : (i + 1) * BL])
        txtT_i = np.roll(txtT_full, -BL * i, axis=1)
        in_maps.append({"imgT": imgT_i, "txtT": txtT_i, "ident": ident})
    return in_maps


def postprocess(results):
    """Host-side gather/reduce of the per-core stats -> (loss, accs)."""
    zrow = np.empty(B, dtype=np.float64)
    rowmax = np.empty(B, dtype=np.float64)
    diag = np.empty(B, dtype=np.float64)
    zcol = np.zeros(B, dtype=np.float64)
    colmax = np.full(B, -np.inf, dtype=np.float64)
    for i, r in enumerate(results):
        # (128, 16*8) f32 / (128, 16, 256) bf16 -> local row index 128*rt + p
        zr = r["zrow"].astype(np.float64).reshape(128, N_RT, N_G).sum(axis=2)
        rm = r["rowmax"].astype(np.float64).max(axis=2)
        zrow[i * BL : (i + 1) * BL] = zr.T.reshape(-1)
        rowmax[i * BL : (i + 1) * BL] = rm.T.reshape(-1)
        diag[i * BL : (i + 1) * BL] = r["diag"].T.reshape(-1).astype(np.float64)
        # (8, 128, 8, 2048): local (rolled) col 2048*g + c; partial over
        # partitions and pair lanes
        cs = r["colsum"].astype(np.float64).sum(axis=(1, 2)).reshape(-1)
        cm = r["colmax"].astype(np.float64).max(axis=(1, 2)).reshape(-1)
        # local col 0 corresponds to global col 2048*i (text was rolled by -2048*i)
        zcol += np.roll(cs, BL * i)
        colmax = np.maximum(colmax, np.roll(cm, BL * i))

    loss_i2t = np.mean(np.log(zrow) - np.log(diag))
    loss_t2i = np.mean(np.log(zcol) - np.log(diag))
    loss = (loss_i2t + loss_t2i) / 2.0
    i2t_acc = np.mean(rowmax == diag)
    t2i_acc = np.mean(colmax == diag)
    return (
        np.float32(loss),
        np.float32(i2t_acc),
        np.float32(t2i_acc),
    )


_program_cache: dict[tuple[float, float], bass.Bass] = {}


def get_program(scale: float, bias: float) -> bass.Bass:
    key = (scale, bias)
    if key not in _program_cache:
        _program_cache[key] = build_program(scale, bias)
    return _program_cache[key]


def compute_scale_bias(image_features, text_features, logit_scale):
    ls = float(np.asarray(logit_scale))
    scale = 100.0 if ls >= math.log(100.0) else float(math.exp(ls))
    # |logits| <= scale * max|img_i| * max|txt_j|; keep exp argument <= ~70
    # so f32 never overflows even for unnormalized inputs.
    img = np.asarray(image_features, dtype=np.float32)
    txt = np.asarray(text_features, dtype=np.float32)
    ni = float(np.sqrt((img.astype(np.float64) ** 2).sum(axis=1).max()))
    nt = float(np.sqrt((txt.astype(np.float64) ** 2).sum(axis=1).max()))
    bound = scale * ni * nt
    bias = -max(0.0, bound - 70.0)
    # fold the fp8 pre-scale into the activation affine
    return scale / (SF * SF), bias


def kernel(image_features, text_features, logit_scale):
    scale, bias = compute_scale_bias(image_features, text_features, logit_scale)
    nc = get_program(scale, bias)
    in_maps = prepare_inputs(image_features, text_features)
    try:
        res = run_bass_kernel_spmd(nc, in_maps, core_ids=list(range(N_CORES)))
    except Exception:
        # transient accelerator hiccups have been observed on this relay;
        # one retry on a fresh attempt usually clears them
        import time as _time

        _time.sleep(2.0)
        res = run_bass_kernel_spmd(nc, in_maps, core_ids=list(range(N_CORES)))
    return postprocess(res.results)
